# revision 2
# baseline (speedup 1.0000x reference)
"""Trainium2 Bass kernel for nn_CAGpool (GNN message passing, CAG pooling).

Sharding: data-parallel over the 64 graph pairs -> 8 pairs (16 component
graphs of 512 nodes) per NeuronCore.  Message passing is dense matmul
against a per-graph 512x512 adjacency built on-device with GPSIMD
local_scatter from host-prepared CSR index lists (index-layout prep only;
all numeric compute happens on device).

Self-loops fold into the adjacency (C+I); the symmetric degree norm is a
src-side per-partition scale on xw plus a dst-side column scale baked
into C once, so each GCN layer is matmuls + one Relu per tile.
"""

import os
import numpy as np
import ml_dtypes

import concourse.bass as bass
import concourse.tile as tile
from concourse import bacc, mybir
from concourse.bass_utils import run_bass_kernel_spmd

F32 = mybir.dt.float32
BF16 = mybir.dt.bfloat16
I16 = mybir.dt.int16

NCORES = 8
B = 64
NPC = B // NCORES          # graph pairs per core (8)
NCG = 2 * NPC              # component graphs per core (16)
N = 512                    # nodes per component graph
K1 = 256
DEBUG = bool(int(os.environ.get("KERNEL_DEBUG", "0")))
STAGE = int(os.environ.get("KERNEL_STAGE", "4"))
SUB = int(os.environ.get("KERNEL_SUB", "99"))


def _layout(ent):
    offs, off = {}, 0
    for nm, w in ent:
        offs[nm] = (off, w)
        off += w
    return offs, off


WOFF, WF_TOT = _layout(
    [("W1", 128), ("W2", 128), ("W3", 128), ("Wgf", 128)]
    + [(f"Wg{i}", 384) for i in range(3)]
    + [(f"Wal{i}", 768) for i in range(6)]
    + [(f"Wf{i}", 128) for i in range(3)]
    + [("Wl1a", 128), ("Wl1b", 128), ("Wl2", 64), ("Wl3", 2),
       ("identb", 128), ("ones", 128), ("brows", 128), ("csel", 256),
       ("rselb", 384), ("rsel", 2048)])
BOFF, BF_TOT = _layout(
    [("bfr", 128), ("balcol", 6), ("bl1col", 1), ("bl2col", 1),
     ("bl3col", 1), ("identf", 128), ("onesf", 128), ("rself", 2048),
     ("bcols", 3)])

_GEOM = {}


def _host_prep(inputs):
    """Build per-core input maps. Index-structure prep only."""
    x = np.asarray(inputs["x"], np.float32)

    s_loc, d_loc = {}, {}
    for comp, (sk, dk) in enumerate((("src_c1", "dst_c1"),
                                     ("src_c2", "dst_c2"))):
        base = (np.arange(B) * N)[:, None]
        s_loc[comp] = np.asarray(inputs[sk]).reshape(B, -1) - base
        d_loc[comp] = np.asarray(inputs[dk]).reshape(B, -1) - base

    # Per (graph, comp): unique (s,d) pairs + multi-edge counts.
    per = {}
    maxw = 2
    for g in range(B):
        for comp in range(2):
            s = s_loc[comp][g]
            d = d_loc[comp][g]
            key = s.astype(np.int64) * N + d.astype(np.int64)
            key = np.concatenate([key, np.arange(N, dtype=np.int64) * (N + 1)])
            uk, cnt = np.unique(key, return_counts=True)
            us = (uk // N).astype(np.int32)
            ud = (uk % N).astype(np.int32)
            per[(g, comp)] = (us, ud, cnt.astype(np.float32))
            w = np.bincount((us & 127) + 128 * (us >> 8), minlength=256).max()
            maxw = max(maxw, int(w))
    IDXW = (maxw + 1) // 2 * 2
    _GEOM["IDXW"] = IDXW

    in_maps = []
    for c in range(NCORES):
        xT = np.empty((128, NCG * N), np.float32)
        sidx = np.full((128, NCG * 2 * IDXW), -1, np.int16)
        sdat = np.zeros((128, NCG * 2 * IDXW), ml_dtypes.bfloat16)
        for comp in range(2):
            for gl in range(NPC):
                g = c * NPC + gl
                cg = comp * NPC + gl
                r0 = g * 2 * N + comp * N
                xT[:, cg * N:(cg + 1) * N] = x[r0:r0 + N].T
                us, ud, cnt = per[(g, comp)]
                sblk = us >> 7
                p = us & 127
                h = sblk >> 1
                idxval = (sblk - 2 * h) * 512 + ud
                for hh in (0, 1):
                    m = h == hh
                    pp, iv, cv = p[m], idxval[m], cnt[m]
                    order = np.argsort(pp, kind="stable")
                    pp, iv, cv = pp[order], iv[order], cv[order]
                    col = np.zeros(len(pp), np.int64)
                    _, sti, cpn = np.unique(pp, return_index=True,
                                            return_counts=True)
                    for si, cn in zip(sti, cpn):
                        col[si:si + cn] = np.arange(cn)
                    base = (cg * 2 + hh) * IDXW
                    sidx[pp, base + col] = iv.astype(np.int16)
                    sdat[pp, base + col] = cv.astype(ml_dtypes.bfloat16)

        wpack = np.zeros((128, WF_TOT), np.float32)

        def put(nm, arr):
            o, w = WOFF[nm]
            arr = np.asarray(arr, np.float32)
            wpack[: arr.shape[0], o:o + arr.shape[1]] = arr

        put("W1", inputs["W1"]); put("W2", inputs["W2"]); put("W3", inputs["W3"])
        put("Wgf", inputs["Wg_fin"])
        for i in range(3):
            put(f"Wg{i}", np.asarray(inputs["Wg_att"])[i * 128:(i + 1) * 128])
        for i in range(6):
            put(f"Wal{i}", np.asarray(inputs["Wal"])[i * 128:(i + 1) * 128])
        for i in range(3):
            put(f"Wf{i}", np.asarray(inputs["Wf"])[i * 128:(i + 1) * 128])
        put("Wl1a", np.asarray(inputs["Wl1"])[:128])
        put("Wl1b", np.asarray(inputs["Wl1"])[128:])
        put("Wl2", inputs["Wl2"])
        put("Wl3", inputs["Wl3"])
        put("identb", np.eye(128, dtype=np.float32))
        put("ones", np.ones((128, 128), np.float32))
        put("brows", np.stack([np.asarray(inputs["b1"]),
                               np.asarray(inputs["b2"]),
                               np.asarray(inputs["b3"])]))
        csel = np.zeros((128, 256), np.float32)
        for cg in range(NCG):
            csel[:, cg * 16 + cg] = 1.0
        put("csel", csel)
        rselb = np.zeros((16, 384), np.float32)
        for l in range(3):
            rselb[l, l * 128:(l + 1) * 128] = 1.0
        put("rselb", rselb)
        rsel = np.zeros((16, 2048), np.float32)
        for cg in range(16):
            rsel[cg, cg * 128:(cg + 1) * 128] = 1.0
        put("rsel", rsel)

        bpack = np.zeros((128, BF_TOT), np.float32)

        def putb(nm, arr):
            o, w = BOFF[nm]
            arr = np.asarray(arr, np.float32)
            bpack[: arr.shape[0], o:o + arr.shape[1]] = arr

        putb("bfr", np.broadcast_to(np.asarray(inputs["bf"])[None, :],
                                    (128, 128)))
        putb("balcol", np.asarray(inputs["bal"]).reshape(6, 128).T)
        putb("bl1col", np.asarray(inputs["bl1"])[:, None])
        putb("bl2col", np.asarray(inputs["bl2"])[:, None])
        putb("bl3col", np.asarray(inputs["bl3"])[:, None])
        putb("bcols", np.stack([np.asarray(inputs["b1"]),
                                np.asarray(inputs["b2"]),
                                np.asarray(inputs["b3"])], 1))
        putb("identf", np.eye(128, dtype=np.float32))
        putb("onesf", np.ones((128, 128), np.float32))
        rself = np.zeros((16, 2048), np.float32)
        for cg in range(16):
            rself[cg, cg * 128:(cg + 1) * 128] = 1.0
        putb("rself", rself)

        in_maps.append({"xT": np.ascontiguousarray(xT), "sidx": sidx,
                        "sdat": sdat, "wpack": wpack, "bpack": bpack})
    return in_maps


def _build(idxw):
    nc = bacc.Bacc("TRN2", target_bir_lowering=False, debug=False,
                   num_devices=NCORES)
    tin = {
        "xT": nc.dram_tensor("xT", [128, NCG * N], F32, kind="ExternalInput"),
        "sidx": nc.dram_tensor("sidx", [128, NCG * 2 * idxw], I16,
                               kind="ExternalInput"),
        "sdat": nc.dram_tensor("sdat", [128, NCG * 2 * idxw], BF16,
                               kind="ExternalInput"),
        "wpack": nc.dram_tensor("wpack", [128, WF_TOT], F32,
                                kind="ExternalInput"),
        "bpack": nc.dram_tensor("bpack", [128, BF_TOT], F32,
                                kind="ExternalInput"),
    }
    t_out = nc.dram_tensor("out", [2, NPC], F32, kind="ExternalOutput")
    dbg = {}
    if DEBUG:
        for nm, shape, dt in (
                ("C", [128, NCG * 2048], BF16), ("deg", [16, N], F32),
                ("xcatT", [128, NCG * 1536], BF16), ("pvT", [128, 48], F32),
                ("scores", [16, N], F32), ("mask", [16, N], F32),
                ("alpha", [16, N], F32), ("gpT", [128, 48], F32),
                ("meanT", [128, 48], F32), ("pab0", [128, N], F32),
                ("scr0", [128, N], F32), ("gpT0", [128, 48], F32),
                ("gpTa", [128, 48], F32), ("gpTb", [128, 48], F32),
                ("hp", [128, NCG * 512], BF16)):
            dbg[nm] = nc.dram_tensor("dbg_" + nm, shape, dt,
                                     kind="ExternalOutput")
    with tile.TileContext(nc, linearize=bool(int(os.environ.get("KERNEL_LINEARIZE", "0")))) as tc:
        _emit(nc, tc, tin, t_out, idxw, dbg)
    nc.compile()
    return nc


def _emit(nc, tc, tin, t_out, idxw, dbg):
    import contextlib
    ctx = contextlib.ExitStack()
    AX = mybir.AxisListType.X
    OP = mybir.AluOpType
    ACT = mybir.ActivationFunctionType

    const = ctx.enter_context(tc.tile_pool(name="const", bufs=1))
    rows = ctx.enter_context(tc.tile_pool(name="rows", bufs=1))
    work = ctx.enter_context(tc.tile_pool(name="work", bufs=2))
    big1 = ctx.enter_context(tc.tile_pool(name="big1", bufs=1))
    ps1 = ctx.enter_context(tc.tile_pool(name="ps1", bufs=1, space="PSUM"))
    ps2 = ctx.enter_context(tc.tile_pool(name="ps2", bufs=2, space="PSUM"))

    wb = const.tile([128, WF_TOT], BF16, tag="wb")
    bp = const.tile([128, BF_TOT], F32, tag="bp")
    xTb = const.tile([128, NCG * N], BF16, tag="xTb")   # reused as hp later
    Call = const.tile([128, NCG * 2048], BF16, tag="Call")
    xcatT = const.tile([128, NCG * 1536], BF16, tag="xcatT")
    rsdcol = const.tile([128, 64], F32, tag="rsdcol")
    mcolf = const.tile([128, 64], F32, tag="mcolf")
    mcolb = const.tile([128, 64], BF16, tag="mcolb")
    mrsd2col = const.tile([128, 64], F32, tag="mrsd2col")
    qcol = const.tile([128, 64], F32, tag="qcol")

    def W(nm):
        o, w = WOFF[nm]
        return wb[:, o:o + w]

    def Bc(nm):
        o, w = BOFF[nm]
        return bp[:, o:o + w]

    nc.gpsimd.dma_start(wb[:], tin["wpack"].ap())      # cast fp32->bf16
    nc.sync.dma_start(bp[:], tin["bpack"].ap())
    nc.gpsimd.dma_start(xTb[:], tin["xT"].ap())        # cast fp32->bf16

    onesb_col = W("ones")[:, 0:1]
    onesb_row = W("ones")[0:1, :]
    onesf_row = Bc("onesf")[0:1, :]
    identb = W("identb")
    identf = Bc("identf")

    def csel(cg):
        o, _ = WOFF["csel"]
        return wb[:, o + cg * 16: o + (cg + 1) * 16]

    def rself(cg):
        o, _ = WOFF["rsel"]
        return wb[0:16, o + cg * 128: o + (cg + 1) * 128]

    def rselb(l):
        o, _ = WOFF["rselb"]
        return wb[0:16, o + l * 128: o + (l + 1) * 128]

    def bcast_row(row_tile, cg, n):
        pb = ps1.tile([128, 512], F32, tag="bcast")
        nc.tensor.matmul(pb[:, :n], lhsT=rself(cg), rhs=row_tile[0:16, 0:n],
                         start=True, stop=True)
        return pb

    # ---- build C -----------------------------------------------------------
    with tc.tile_pool(name="edges", bufs=1) as epool:
        sidx = epool.tile([128, NCG * 2 * idxw], I16, tag="sidx")
        sdat = epool.tile([128, NCG * 2 * idxw], BF16, tag="sdat")
        nc.sync.dma_start(sidx[:], tin["sidx"].ap())
        nc.sync.dma_start(sdat[:], tin["sdat"].ap())
        for cg in range(NCG):
            for h in (0, 1):
                b0 = (cg * 2 + h) * idxw
                nc.gpsimd.local_scatter(
                    Call[:, cg * 2048 + h * 1024: cg * 2048 + (h + 1) * 1024],
                    sdat[:, b0:b0 + idxw], sidx[:, b0:b0 + idxw],
                    channels=128, num_elems=1024, num_idxs=idxw)

    # ---- degree rows -------------------------------------------------------
    ps_deg = ps1.tile([16, N], F32, tag="stat")
    for cg in range(NCG):
        for sblk in range(4):
            nc.tensor.matmul(
                ps_deg[:], lhsT=csel(cg),
                rhs=Call[:, cg * 2048 + sblk * 512: cg * 2048 + (sblk + 1) * 512],
                start=(cg == 0 and sblk == 0),
                stop=(cg == NCG - 1 and sblk == 3))
    deg_row = rows.tile([16, N], F32, tag="deg")
    nc.vector.tensor_copy(deg_row[:], ps_deg[:])
    sq_row = rows.tile([16, N], F32, tag="sq")
    nc.scalar.activation(sq_row[:], deg_row[:], ACT.Sqrt)
    rsd_row = rows.tile([16, N], F32, tag="rsd")
    nc.vector.reciprocal(rsd_row[:], sq_row[:])
    if DEBUG:
        nc.sync.dma_start(dbg["deg"].ap(), deg_row[:])

    for sblk in range(4):
        pt = ps1.tile([128, 128], F32, tag="bcast")
        nc.tensor.transpose(pt[:, 0:16], rsd_row[:, sblk * 128:(sblk + 1) * 128],
                            identf[0:16, 0:16])
        nc.vector.tensor_copy(rsdcol[:, sblk * 16:(sblk + 1) * 16], pt[:, 0:16])

    # ---- fold dst-side norm into C ----------------------------------------
    rsd_rowb = rows.tile([16, N], BF16, tag="rsdb")
    nc.vector.tensor_copy(rsd_rowb[:], rsd_row[:])
    for cg in range(NCG):
        pb = bcast_row(rsd_rowb, cg, N)
        for sblk in range(4):
            sl = Call[:, cg * 2048 + sblk * 512: cg * 2048 + (sblk + 1) * 512]
            nc.vector.tensor_tensor(sl, sl, pb[:], op=OP.mult)
            nc.scalar.mul(sl, sl, rsdcol[:, sblk * 16 + cg: sblk * 16 + cg + 1])
    if DEBUG:
        nc.sync.dma_start(dbg["C"].ap(), Call[:])

    # ---- 3 GCN layers ------------------------------------------------------
    if STAGE < 2:
        o3 = rows.tile([2, NPC], F32, tag="o3")
        nc.vector.memset(o3[:], 0.0)
        nc.sync.dma_start(t_out.ap(), o3[:])
        ctx.close()
        return
    for l in range(3):
        wl = W(("W1", "W2", "W3")[l])
        bcol = Bc("bcols")[:, l:l + 1]
        for cg in range(NCG):
            xws = work.tile([128, 512], BF16, tag="xws")
            pxw = ps2.tile([128, 512], F32, tag="mmw")
            for nt in range(4):
                if l == 0:
                    lhsT = xTb[:, cg * N + nt * 128: cg * N + (nt + 1) * 128]
                else:
                    lhsT = xcatT[:, cg * 1536 + (l - 1) * 512 + nt * 128:
                                 cg * 1536 + (l - 1) * 512 + (nt + 1) * 128]
                nc.tensor.matmul(pxw[:, nt * 128:(nt + 1) * 128], lhsT=lhsT,
                                 rhs=wl, start=True, stop=True)
            nc.scalar.activation(xws[:], pxw[:], ACT.Copy)
            ph = ps2.tile([128, 512], F32, tag="mmw")
            for sblk in range(4):
                nc.tensor.matmul(
                    ph[:],
                    lhsT=xws[:, sblk * 128:(sblk + 1) * 128],
                    rhs=Call[:, cg * 2048 + sblk * 512:
                             cg * 2048 + (sblk + 1) * 512],
                    start=(sblk == 0), stop=(sblk == 3))
            nc.scalar.activation(
                xcatT[:, cg * 1536 + l * 512: cg * 1536 + (l + 1) * 512],
                ph[:], ACT.Relu, bias=bcol)
    if DEBUG:
        nc.sync.dma_start(dbg["xcatT"].ap(), xcatT[:])

    # ---- attention pool + att_lin -----------------------------------------
    if STAGE < 3:
        o3 = rows.tile([2, NPC], F32, tag="o3")
        nc.vector.memset(o3[:], 0.0)
        nc.sync.dma_start(t_out.ap(), o3[:])
        ctx.close()
        return
    meanT = rows.tile([128, 48], F32, tag="meanT")
    for cg in range(NCG):
        for ch in range(3):
            sl = xcatT[:, cg * 1536 + ch * 512: cg * 1536 + (ch + 1) * 512]
            nc.vector.tensor_reduce(meanT[:, ch * 16 + cg: ch * 16 + cg + 1],
                                    sl, axis=AX, op=OP.add)
    meanTb = rows.tile([128, 48], BF16, tag="meanTb")
    nc.scalar.activation(meanTb[:], meanT[:], ACT.Copy, scale=1.0 / N)
    cT = rows.tile([128, 48], F32, tag="cT")
    for fo in range(3):
        pc = ps2.tile([128, 128], F32, tag="mm")
        for fi in range(3):
            nc.tensor.matmul(pc[:, 0:16],
                             lhsT=W(f"Wg{fi}")[:, fo * 128:(fo + 1) * 128],
                             rhs=meanTb[:, fi * 16:(fi + 1) * 16],
                             start=(fi == 0), stop=(fi == 2))
        nc.scalar.activation(cT[:, fo * 16:(fo + 1) * 16], pc[:, 0:16],
                             ACT.Tanh)

    ps_al = ps1.tile([16, N], F32, tag="stat")
    for cg in range(NCG):
        for ch in range(3):
            mlh = work.tile([128, 16], BF16, tag="mlh")
            nc.vector.tensor_scalar(
                mlh[:], csel(cg), cT[:, ch * 16 + cg: ch * 16 + cg + 1], None,
                op0=OP.mult)
            nc.tensor.matmul(
                ps_al[:], lhsT=mlh[:],
                rhs=xcatT[:, cg * 1536 + ch * 512: cg * 1536 + (ch + 1) * 512],
                start=(cg == 0 and ch == 0),
                stop=(cg == NCG - 1 and ch == 2))
    alpha_row = rows.tile([16, N], BF16, tag="alpha")
    nc.scalar.activation(alpha_row[:], ps_al[:], ACT.Sigmoid)
    if DEBUG:
        nc.sync.dma_start(dbg["alpha"].ap(), alpha_row[:])
        nc.sync.dma_start(dbg["meanT"].ap(), meanT[:])

    if SUB < 2:
        o3 = rows.tile([2, NPC], F32, tag="o3")
        nc.vector.memset(o3[:], 0.0)
        nc.sync.dma_start(t_out.ap(), o3[:])
        ctx.close()
        return
    gpT = rows.tile([128, 48], F32, tag="gpT")
    for cg in range(NCG):
        pab = bcast_row(alpha_row, cg, N)
        for ch in range(3):
            scr = work.tile([128, 512], BF16, tag="scr")
            nc.vector.tensor_tensor(
                scr[:],
                xcatT[:, cg * 1536 + ch * 512: cg * 1536 + (ch + 1) * 512],
                pab[:], op=OP.mult)
            nc.vector.tensor_reduce(gpT[:, ch * 16 + cg: ch * 16 + cg + 1],
                                    scr[:], axis=AX, op=OP.add)
            if DEBUG and cg == 0 and ch == 1:
                nc.sync.dma_start(dbg["gpTa"].ap(), gpT[:])
            if DEBUG and cg == 1 and ch == 0:
                nc.sync.dma_start(dbg["gpTb"].ap(), gpT[:])
            if DEBUG and cg == 0 and ch == 0:
                pabc = work.tile([128, N], F32, tag="pabc")
                nc.vector.tensor_copy(pabc[:], pab[:])
                nc.sync.dma_start(dbg["pab0"].ap(), pabc[:])
                nc.sync.dma_start(dbg["scr0"].ap(), scr[:])
                nc.sync.dma_start(dbg["gpT0"].ap(), gpT[:])

    if SUB < 12:
        o3 = rows.tile([2, NPC], F32, tag="o3")
        nc.vector.memset(o3[:], 0.0)
        nc.sync.dma_start(t_out.ap(), o3[:])
        ctx.close()
        return
    if DEBUG:
        nc.sync.dma_start(dbg["gpT"].ap(), gpT[:])
    gpcatTb = rows.tile([128, 48], BF16, tag="gpcatTb")
    for j in range(6):
        comp, ch = j // 3, j % 3
        nc.vector.tensor_copy(
            gpcatTb[:, j * 8:(j + 1) * 8],
            gpT[:, ch * 16 + comp * 8: ch * 16 + comp * 8 + 8])
    pvTb = rows.tile([128, 48], BF16, tag="pvTb")
    pvTf = rows.tile([128, 48], F32, tag="pvTf")
    for co in range(6):
        pp = ps2.tile([128, 128], F32, tag="mm")
        for ci in range(6):
            nc.tensor.matmul(pp[:, 0:8],
                             lhsT=W(f"Wal{ci}")[:, co * 128:(co + 1) * 128],
                             rhs=gpcatTb[:, ci * 8:(ci + 1) * 8],
                             start=(ci == 0), stop=(ci == 5))
        nc.vector.tensor_scalar(pvTf[:, co * 8:(co + 1) * 8], pp[:, 0:8],
                                Bc("balcol")[:, co:co + 1], None, op0=OP.add)
        nc.vector.tensor_copy(pvTb[:, co * 8:(co + 1) * 8],
                              pvTf[:, co * 8:(co + 1) * 8])
    if DEBUG:
        nc.sync.dma_start(dbg["pvT"].ap(), pvTf[:])

    if SUB < 13:
        o3 = rows.tile([2, NPC], F32, tag="o3")
        nc.vector.memset(o3[:], 0.0)
        nc.sync.dma_start(t_out.ap(), o3[:])
        ctx.close()
        return
    rsncol = rows.tile([16, 1], F32, tag="rsncol")
    pn = ps2.tile([128, 512], F32, tag="mm")
    for ci in range(6):
        comp = ci // 3
        mpv = work.tile([128, 16], BF16, tag="mpv")
        nc.vector.memset(mpv[:], 0.0)
        nc.vector.tensor_copy(mpv[:, comp * 8:(comp + 1) * 8],
                              pvTb[:, ci * 8:(ci + 1) * 8])
        nc.tensor.matmul(pn[0:16, 0:16], lhsT=mpv[:], rhs=mpv[:],
                         start=(ci == 0), stop=(ci == 5))
    dd = work.tile([16, 16], F32, tag="dd")
    nc.vector.tensor_tensor(dd[:], pn[0:16, 0:16], identf[0:16, 0:16],
                            op=OP.mult)
    nn = work.tile([16, 1], F32, tag="nn")
    nc.vector.tensor_reduce(nn[:], dd[:], axis=AX, op=OP.add)
    sqn = work.tile([16, 1], F32, tag="sqn")
    nc.scalar.activation(sqn[:], nn[:], ACT.Sqrt)
    nc.vector.reciprocal(rsncol[:], sqn[:])

    if SUB < 14:
        o3 = rows.tile([2, NPC], F32, tag="o3")
        nc.vector.memset(o3[:], 0.0)
        nc.sync.dma_start(t_out.ap(), o3[:])
        ctx.close()
        return
    ps_sc = ps1.tile([16, N], F32, tag="stat")
    for cg in range(NCG):
        comp, g = cg // NPC, cg % NPC
        for ci in range(3):
            mlh = work.tile([128, 16], BF16, tag="mlh")
            nc.vector.tensor_scalar(
                mlh[:], csel(cg),
                pvTf[:, (comp * 3 + ci) * 8 + g:(comp * 3 + ci) * 8 + g + 1],
                None, op0=OP.mult)
            nc.tensor.matmul(
                ps_sc[:], lhsT=mlh[:],
                rhs=xcatT[:, cg * 1536 + ci * 512: cg * 1536 + (ci + 1) * 512],
                start=(cg == 0 and ci == 0),
                stop=(cg == NCG - 1 and ci == 2))
    score_row = rows.tile([16, N], F32, tag="score")
    nc.scalar.activation(score_row[:], ps_sc[:], ACT.Copy, scale=rsncol[:])
    if DEBUG:
        nc.sync.dma_start(dbg["scores"].ap(), score_row[:])

    if SUB < 3:
        o3 = rows.tile([2, NPC], F32, tag="o3")
        nc.vector.memset(o3[:], 0.0)
        nc.sync.dma_start(t_out.ap(), o3[:])
        ctx.close()
        return
    # ---- top-256 mask (32 rounds of max8 + match_replace) -----------------
    cur = rows.tile([16, N], F32, tag="cur")
    nc.vector.tensor_copy(cur[:], score_row[:])
    mx = rows.tile([16, 8], F32, tag="mx")
    for _ in range(K1 // 8):
        nc.vector.max(out=mx[:], in_=cur[:])
        nc.vector.match_replace(out=cur[:], in_to_replace=mx[:],
                                in_values=cur[:], imm_value=-1e30)
    mask_row = rows.tile([16, N], F32, tag="mask")
    nc.vector.tensor_tensor(mask_row[:], score_row[:], cur[:], op=OP.not_equal)
    if DEBUG:
        nc.sync.dma_start(dbg["mask"].ap(), mask_row[:])
    if SUB < 4:
        o3 = rows.tile([2, NPC], F32, tag="o3")
        nc.vector.memset(o3[:], 0.0)
        nc.sync.dma_start(t_out.ap(), o3[:])
        ctx.close()
        return
    sig_row = rows.tile([16, N], F32, tag="sig")
    nc.scalar.activation(sig_row[:], score_row[:], ACT.Sigmoid)
    gate_row = rows.tile([16, N], BF16, tag="gate")
    nc.vector.tensor_tensor(gate_row[:], sig_row[:], mask_row[:], op=OP.mult)

    for sblk in range(4):
        pt = ps1.tile([128, 128], F32, tag="bcast")
        nc.tensor.transpose(pt[:, 0:16],
                            mask_row[:, sblk * 128:(sblk + 1) * 128],
                            identf[0:16, 0:16])
        nc.vector.tensor_copy(mcolf[:, sblk * 16:(sblk + 1) * 16], pt[:, 0:16])
        nc.vector.tensor_copy(mcolb[:, sblk * 16:(sblk + 1) * 16], pt[:, 0:16])

    # ---- pooled degree -----------------------------------------------------
    if STAGE < 4:
        o3 = rows.tile([2, NPC], F32, tag="o3")
        nc.vector.memset(o3[:], 0.0)
        nc.sync.dma_start(t_out.ap(), o3[:])
        ctx.close()
        return
    sqcol = const.tile([128, 64], F32, tag="sqcol")
    for sblk in range(4):
        pt = ps1.tile([128, 128], F32, tag="bcast")
        nc.tensor.transpose(pt[:, 0:16], sq_row[:, sblk * 128:(sblk + 1) * 128],
                            identf[0:16, 0:16])
        nc.vector.tensor_copy(sqcol[:, sblk * 16:(sblk + 1) * 16], pt[:, 0:16])
    msqcol = const.tile([128, 64], F32, tag="msqcol")
    nc.vector.tensor_tensor(msqcol[:], mcolf[:], sqcol[:], op=OP.mult)
    ps_d2 = ps1.tile([16, N], F32, tag="stat")
    for cg in range(NCG):
        for sblk in range(4):
            mlh = work.tile([128, 16], BF16, tag="mlh")
            nc.vector.tensor_scalar(
                mlh[:], csel(cg),
                msqcol[:, sblk * 16 + cg: sblk * 16 + cg + 1], None,
                op0=OP.mult)
            nc.tensor.matmul(
                ps_d2[:], lhsT=mlh[:],
                rhs=Call[:, cg * 2048 + sblk * 512: cg * 2048 + (sblk + 1) * 512],
                start=(cg == 0 and sblk == 0),
                stop=(cg == NCG - 1 and sblk == 3))
    deg2_row = rows.tile([16, N], F32, tag="deg2")
    nc.vector.tensor_tensor(deg2_row[:], ps_d2[:], mask_row[:], op=OP.mult)
    nc.vector.tensor_tensor(deg2_row[:], deg2_row[:], sq_row[:], op=OP.mult)
    nc.vector.tensor_tensor(deg2_row[:], deg2_row[:], mask_row[:],
                            op=OP.subtract)
    nc.vector.tensor_scalar(deg2_row[:], deg2_row[:], 1.0, None, op0=OP.add)
    sq2_row = rows.tile([16, N], F32, tag="sq2")
    nc.scalar.activation(sq2_row[:], deg2_row[:], ACT.Sqrt)
    rsd2_row = rows.tile([16, N], F32, tag="rsd2")
    nc.vector.reciprocal(rsd2_row[:], sq2_row[:])
    mrsd2_row = rows.tile([16, N], F32, tag="mrsd2")
    nc.vector.tensor_tensor(mrsd2_row[:], rsd2_row[:], mask_row[:], op=OP.mult)
    q_row = rows.tile([16, N], F32, tag="qrow")
    nc.vector.tensor_tensor(q_row[:], mrsd2_row[:], sq_row[:], op=OP.mult)
    for sblk in range(4):
        pt = ps1.tile([128, 128], F32, tag="bcast")
        nc.tensor.transpose(pt[:, 0:16],
                            mrsd2_row[:, sblk * 128:(sblk + 1) * 128],
                            identf[0:16, 0:16])
        nc.vector.tensor_copy(mrsd2col[:, sblk * 16:(sblk + 1) * 16],
                              pt[:, 0:16])
        pt2 = ps1.tile([128, 128], F32, tag="bcast")
        nc.tensor.transpose(pt2[:, 0:16], q_row[:, sblk * 128:(sblk + 1) * 128],
                            identf[0:16, 0:16])
        nc.vector.tensor_copy(qcol[:, sblk * 16:(sblk + 1) * 16], pt2[:, 0:16])

    # ---- pooled conv + final attention pool -------------------------------
    hpall = xTb  # reuse (xTb fully consumed by layer 1)
    ps_mT = ps1.tile([128, 16], F32, tag="mT2")
    for cg in range(NCG):
        pgb = bcast_row(gate_row, cg, N)
        pT = big1.tile([128, 1536], BF16, tag="pT")
        for ch in range(3):
            nc.vector.tensor_tensor(
                pT[:, ch * 512:(ch + 1) * 512],
                xcatT[:, cg * 1536 + ch * 512: cg * 1536 + (ch + 1) * 512],
                pgb[:], op=OP.mult)
        xwps = work.tile([128, 512], BF16, tag="xwps")
        pxp = ps2.tile([128, 512], F32, tag="mmw")
        for nt in range(4):
            for ci in range(3):
                nc.tensor.matmul(
                    pxp[:, nt * 128:(nt + 1) * 128],
                    lhsT=pT[:, ci * 512 + nt * 128: ci * 512 + (nt + 1) * 128],
                    rhs=W(f"Wf{ci}"), start=(ci == 0), stop=(ci == 2))
        for nt in range(4):
            nc.scalar.activation(
                xwps[:, nt * 128:(nt + 1) * 128],
                pxp[:, nt * 128:(nt + 1) * 128], ACT.Copy,
                scale=qcol[:, nt * 16 + cg: nt * 16 + cg + 1])
        hp = hpall[:, cg * 512:(cg + 1) * 512]
        for dt in range(4):
            pm = ps2.tile([128, 128], F32, tag="mm")
            for sblk in range(4):
                nc.tensor.matmul(
                    pm[:],
                    lhsT=Call[:, cg * 2048 + sblk * 512 + dt * 128:
                              cg * 2048 + sblk * 512 + (dt + 1) * 128],
                    rhs=xwps[:, sblk * 128:(sblk + 1) * 128],
                    start=(sblk == 0), stop=(sblk == 3))
            tmp = work.tile([128, 128], F32, tag="tmp")
            nc.scalar.activation(tmp[:], pm[:], ACT.Copy,
                                 scale=qcol[:, dt * 16 + cg: dt * 16 + cg + 1])
            nc.vector.tensor_tensor(tmp[:], tmp[:], Bc("bfr"), op=OP.add)
            nc.scalar.activation(hp[:, dt * 128:(dt + 1) * 128], tmp[:],
                                 ACT.Relu,
                                 scale=mcolf[:, dt * 16 + cg: dt * 16 + cg + 1])
        for dt in range(4):
            nc.tensor.matmul(ps_mT[:, cg:cg + 1],
                             lhsT=hp[:, dt * 128:(dt + 1) * 128],
                             rhs=onesb_col, start=(dt == 0), stop=(dt == 3))
    if DEBUG:
        nc.sync.dma_start(dbg["hp"].ap(), hpall[:])

    mT2b = rows.tile([128, 16], BF16, tag="mT2b")
    nc.scalar.activation(mT2b[:], ps_mT[:], ACT.Copy, scale=1.0 / K1)
    pc2 = ps2.tile([128, 128], F32, tag="mm")
    nc.tensor.matmul(pc2[:, 0:16], lhsT=W("Wgf"), rhs=mT2b[:], start=True,
                     stop=True)
    c2Tf = rows.tile([128, 16], F32, tag="c2Tf")
    nc.scalar.activation(c2Tf[:], pc2[:, 0:16], ACT.Tanh)
    ptc = ps1.tile([128, 128], F32, tag="bcast")
    nc.tensor.transpose(ptc[0:16, :], c2Tf[:], identf)
    c2rows = rows.tile([16, 128], BF16, tag="c2rows")
    nc.vector.tensor_copy(c2rows[:], ptc[0:16, :])

    ps_g = ps1.tile([128, 16], F32, tag="gfin")
    for cg in range(NCG):
        pcb = bcast_row(c2rows, cg, 128)
        apre = work.tile([128, 4], F32, tag="apre")
        hp = hpall[:, cg * 512:(cg + 1) * 512]
        for dt in range(4):
            scr2 = work.tile([128, 128], F32, tag="scr2")
            nc.vector.tensor_tensor(scr2[:], hp[:, dt * 128:(dt + 1) * 128],
                                    pcb[:, 0:128], op=OP.mult)
            nc.vector.tensor_reduce(apre[:, dt:dt + 1], scr2[:], axis=AX,
                                    op=OP.add)
        a4 = work.tile([128, 4], BF16, tag="a4")
        nc.scalar.activation(a4[:], apre[:], ACT.Sigmoid)
        for dt in range(4):
            nc.tensor.matmul(ps_g[:, cg:cg + 1],
                             lhsT=hp[:, dt * 128:(dt + 1) * 128],
                             rhs=a4[:, dt:dt + 1], start=(dt == 0),
                             stop=(dt == 3))

    pcat = rows.tile([128, 16], BF16, tag="pcat")
    nc.vector.tensor_copy(pcat[:], ps_g[:])
    p1 = ps2.tile([128, 128], F32, tag="mm")
    nc.tensor.matmul(p1[:, 0:NPC], lhsT=W("Wl1a"), rhs=pcat[:, 0:NPC],
                     start=True, stop=False)
    nc.tensor.matmul(p1[:, 0:NPC], lhsT=W("Wl1b"), rhs=pcat[:, NPC:2 * NPC],
                     start=False, stop=True)
    o1 = rows.tile([128, NPC], BF16, tag="o1")
    nc.scalar.activation(o1[:], p1[:, 0:NPC], ACT.Relu, bias=Bc("bl1col")[:])
    p2 = ps2.tile([128, 128], F32, tag="mm")
    nc.tensor.matmul(p2[0:64, 0:NPC], lhsT=W("Wl2"), rhs=o1[:], start=True,
                     stop=True)
    o2 = rows.tile([64, NPC], BF16, tag="o2")
    nc.scalar.activation(o2[:], p2[0:64, 0:NPC], ACT.Relu,
                         bias=Bc("bl2col")[0:64, :])
    p3 = ps2.tile([128, 128], F32, tag="mm")
    nc.tensor.matmul(p3[0:2, 0:NPC], lhsT=W("Wl3")[0:64, :], rhs=o2[:],
                     start=True, stop=True)
    o3 = rows.tile([2, NPC], F32, tag="o3")
    nc.vector.tensor_scalar(o3[:], p3[0:2, 0:NPC], Bc("bl3col")[0:2, :],
                            None, op0=OP.add)
    nc.sync.dma_start(t_out.ap(), o3[:])
    ctx.close()


_NC_CACHE = {}


def _get_nc(idxw):
    key = (idxw, STAGE, SUB, DEBUG)
    if key not in _NC_CACHE:
        _NC_CACHE[key] = _build(idxw)
    return _NC_CACHE[key]


def kernel(**inputs):
    in_maps = _host_prep(inputs)
    nc = _get_nc(_GEOM["IDXW"])
    trace = bool(int(os.environ.get("KERNEL_TRACE", "0")))
    res = run_bass_kernel_spmd(nc, in_maps, core_ids=list(range(NCORES)),
                               trace=trace)
    out = np.empty((B, 2), np.float32)
    for c in range(NCORES):
        out[c * NPC:(c + 1) * NPC] = res.results[c]["out"].T
    kernel._last = res
    return out

